# revision 37
# baseline (speedup 1.0000x reference)
"""CLIP ViT-B/16 vision transformer forward pass on 8 Trainium2 NeuronCores.

Strategy: data-parallel over the batch (32 images -> 4 per core), weights
replicated. One SPMD Bass/Tile program runs on all 8 cores; host shards
inputs, gathers [4,1000] logits per core into [32,1000].

Layout scheme inside a core (B=4 images, 197 tokens, D=768):
  - residual stream x: token-major f32 SBUF [128, 8, 768]; token tile t
    holds image t//2, tokens 128*(t%2) .. (valid rows: 128 / 69).
  - q/k/v and attn-proj matmuls run in fp8e4 DoubleRow (f32 PSUM
    accumulate) with power-of-2 scales folded into the PSUM->SBUF copies;
    attention internals and the MLP stay bf16 for accuracy (fp8 on
    fc/cproj measured 3e-2 rel err vs the 9e-3 shipped). LayerNorm
    scales/biases are folded into the adjacent weight matrices on the
    host (exact).
  - attention computes TRANSPOSED scores St = K^T-padded @ Q directly
    (zero-padded K as stationary), so softmax(A)^T needed by the O=V^T A^T
    matmul comes straight from the Exp activation -- no PE transposes and
    no PSUM->SBUF copies. Softmax denominators come from ones-column
    matmuls into the same PSUM tile (replicated across partitions), and
    the normalization is fused into the O PSUM->SBUF copy.
"""

from contextlib import ExitStack

import numpy as np
import ml_dtypes

import concourse.bass as bass
import concourse.tile as tile
import concourse.mybir as mybir
from concourse import bacc
from concourse.bass_utils import run_bass_kernel_spmd
from concourse.masks import make_identity

BF = mybir.dt.bfloat16
F32 = mybir.dt.float32
FP8 = mybir.dt.float8e4
bf16 = ml_dtypes.bfloat16
f8e4 = ml_dtypes.float8_e4m3
AF = mybir.ActivationFunctionType
DR = mybir.MatmulPerfMode.DoubleRow

P = 128
D = 768
KD = 6            # D / 128
H = 12
DH = 64
NTOK = 197
IMGS = 4          # images per core
NT = 2 * IMGS     # token tiles per core
T = IMGS * NTOK   # 788 feature-major token columns
TP8 = 800         # fp8 h free-dim padded so pair-stride % 16 == 0
MLP = 3072
KM = 24           # MLP / 128
DEPTH = 12
PROJ = 512
NCLS = 1000
EPS = 1e-5
N_CORES = 8
TN = 394          # T / 2, qkv/fc moving-chunk
DN = 384          # D / 2, tm-output free chunk
SH = 8.0          # fp8 activation scale (LN output, bounded by sqrt(767)<30)
SO = 64.0         # fp8 attention-output scale (|O| <= max|V| ~ 2.5)
NT8 = 208         # fp8 o free-dim padded so pair-stride % 16 == 0


def _tile_geom(t):
    """token tile -> (img, col offset within image, valid rows)"""
    b, h = divmod(t, 2)
    return b, 128 * h, (128 if h == 0 else NTOK - 128)


class _Ctx:
    """Shared build state: nc, pools, constants."""


def _layernorm_tile(c, src_ap, dst_ap, w):
    """dst = (src - mean)*rstd over last dim (768), rows [0:w)."""
    nc = c.nc
    stats = c.small.tile([P, 3, 6], F32, tag="stats")
    for s in range(3):
        nc.vector.bn_stats(stats[:w, s, :], src_ap[:, 256 * s:256 * (s + 1)])
    mv = c.small.tile([P, 2], F32, tag="mv")
    nc.vector.bn_aggr(mv[:w], stats[:w])
    sd = c.small.tile([P, 1], F32, tag="sd")
    nc.scalar.activation(sd[:w], mv[:w, 1:2], AF.Sqrt, bias=c.epst[:w])
    nc.vector.reciprocal(sd[:w], sd[:w])
    nc.vector.tensor_scalar(
        out=dst_ap, in0=src_ap, scalar1=mv[:w, 0:1], scalar2=sd[:w],
        op0=mybir.AluOpType.subtract, op1=mybir.AluOpType.mult)


def _ln_transpose(c, h_fm, scale):
    """LN over x_tm then feature-major transpose into h_fm.

    h_fm is [P, KD, *] (fp8 with scale=SH, or bf16 with scale=None).
    All 8 tiles' stats are computed first so ONE Sqrt serves the whole
    pass (the activation-table reload for Sqrt<->Exp<->Gelu is ~1.3us a
    pop and the scheduler otherwise ping-pongs them); transposes are
    batched 4-to-a-PSUM-bank so each copy moves 4 chunks.
    """
    nc = c.nc
    mvs = c.small.tile([P, NT, 2], F32, tag="mvs")
    nc.vector.memset(mvs, 1.0)
    sds = c.small.tile([P, NT], F32, tag="sds")
    for g in range(2):
        for t in range(4 * g, 4 * g + 4):
            _, _, w = _tile_geom(t)
            stats = c.small.tile([P, 2, 6], F32, tag="stats")
            for s in range(2):
                nc.vector.bn_stats(stats[:w, s, :],
                                   c.x_tm[t][:w, 384 * s:384 * (s + 1)])
            nc.vector.bn_aggr(mvs[:w, t, :], stats[:w])
        nc.scalar.activation(sds[:, 4 * g:4 * g + 4],
                             mvs[:, 4 * g:4 * g + 4, 1], AF.Sqrt, bias=c.epst)
        nc.vector.reciprocal(sds[:, 4 * g:4 * g + 4], sds[:, 4 * g:4 * g + 4])
    for t in range(NT):
        b, ci, w = _tile_geom(t)
        gc = NTOK * b + ci
        ht = c.htp.tile([P, D], BF, tag="ht")
        nc.vector.tensor_scalar(
            out=ht[:w], in0=c.x_tm[t][:w, :], scalar1=mvs[:w, t, 0:1],
            scalar2=sds[:w, t:t + 1],
            op0=mybir.AluOpType.subtract, op1=mybir.AluOpType.mult)
        for g, nk in ((0, 4), (4, 2)):
            tp = c.tpp.tile([P, 4, P], BF, tag="tp")
            for j in range(nk):
                kc = g + j
                nc.tensor.transpose(tp[:P, j, :w], ht[:w, P * kc:P * (kc + 1)],
                                    c.idb[:w, :w])
            # split the PSUM drains across ACT and DVE so neither starves PE
            if scale is None:
                if g == 0:
                    nc.scalar.copy(h_fm[:, g:g + nk, gc:gc + w],
                                   tp[:, :nk, :w])
                else:
                    nc.vector.tensor_copy(h_fm[:, g:g + nk, gc:gc + w],
                                          tp[:, :nk, :w])
            else:
                if g == 0:
                    nc.scalar.mul(h_fm[:, g:g + nk, gc:gc + w],
                                  tp[:, :nk, :w], scale)
                else:
                    nc.vector.tensor_scalar_mul(h_fm[:, g:g + nk, gc:gc + w],
                                                tp[:, :nk, :w], scale)


def _embed(c, patches_d, wconv_d, pos_d):
    nc = c.nc
    with tc_pool(c, "embed", 1) as emb, tc_pool(c, "pimg", 2) as pimg:
        wconv_sb = emb.tile([P, KD, D], BF)
        nc.sync.dma_start(wconv_sb, wconv_d.ap())
        pos_sb = emb.tile([P, 2, D], F32)
        nc.sync.dma_start(pos_sb, pos_d.ap())
        for b in range(IMGS):
            pa = pimg.tile([P, KD, NTOK], BF, tag="pa")
            nc.sync.dma_start(pa, patches_d.ap()[:, :, b, :])
            for half in range(2):
                t = 2 * b + half
                _, ci, w = _tile_geom(t)
                for n2 in range(2):
                    ps = c.mmp.tile([P, 512], F32, tag="mm")
                    for kc in range(KD):
                        nc.tensor.matmul(
                            ps[:w, :DN],
                            lhsT=pa[:, kc, ci:ci + w],
                            rhs=wconv_sb[:, kc, DN * n2:DN * (n2 + 1)],
                            start=(kc == 0), stop=(kc == KD - 1))
                    nc.vector.tensor_add(
                        c.x_tm[t][:w, DN * n2:DN * (n2 + 1)], ps[:w, :DN],
                        pos_sb[:w, half, DN * n2:DN * (n2 + 1)])
        for t in range(NT):
            _, _, w = _tile_geom(t)
            _layernorm_tile(c, c.x_tm[t][:w, :], c.x_tm[t][:w, :], w)


def _qk_phase(c, wqk_sb, h8, q_fm, dsq):
    """Q = h @ Wq in fp8 DoubleRow; descale folded into the ACT copies."""
    nc = c.nc
    for m in range(KD):
        for n2 in range(2):
            ps = c.mmp.tile([P, 512], F32, tag="mm")
            for kc in range(0, KD, 2):
                nc.tensor.matmul(
                    ps[:P, :TN],
                    lhsT=wqk_sb[:, kc:kc + 2, P * m:P * (m + 1)],
                    rhs=h8[:, kc:kc + 2, TN * n2:TN * (n2 + 1)],
                    start=(kc == 0), stop=(kc == KD - 2), perf_mode=DR)
            if n2 == 0:
                nc.scalar.mul(q_fm[:, m, TN * n2:TN * (n2 + 1)],
                              ps[:P, :TN], dsq)
            else:
                nc.vector.tensor_scalar_mul(q_fm[:, m, TN * n2:TN * (n2 + 1)],
                                            ps[:P, :TN], dsq)


def _k_phase(c, b, wqk_sb, h8, k_z, dsk):
    """K for one image into zero-padded per-head layout [128, 12, 197]:
    head h lives at partitions 64*(h%2) .. +64, other half is zero (the
    zero halves are written once at kernel start and never touched)."""
    nc = c.nc
    for m in range(KD):
        ps = c.mmp.tile([P, 512], F32, tag="mm")
        for kc in range(0, KD, 2):
            nc.tensor.matmul(
                ps[:P, :NTOK],
                lhsT=wqk_sb[:, kc:kc + 2, P * (KD + m):P * (KD + m + 1)],
                rhs=h8[:, kc:kc + 2, NTOK * b:NTOK * (b + 1)],
                start=(kc == 0), stop=(kc == KD - 2), perf_mode=DR)
        nc.scalar.mul(k_z[0:64, 2 * m, :], ps[0:64, :NTOK], dsk)
        nc.vector.tensor_scalar_mul(k_z[64:128, 2 * m + 1, :],
                                    ps[64:128, :NTOK], dsk)


def _v_phase(c, b, wv_sb, h8, v_sb, dsv):
    nc = c.nc
    if c.dbg:
        nc.vector.memset(v_sb, 0.0)
    for half in range(2):
        t = 2 * b + half
        _, ci, w = _tile_geom(t)
        gc = NTOK * b + ci
        for n2 in range(2):
            ps = c.mmp.tile([P, 512], F32, tag="mm")
            for kc in range(0, KD, 2):
                nc.tensor.matmul(
                    ps[:w, :DN],
                    lhsT=h8[:, kc:kc + 2, gc:gc + w],
                    rhs=wv_sb[:, kc:kc + 2, DN * n2:DN * (n2 + 1)],
                    start=(kc == 0), stop=(kc == KD - 2), perf_mode=DR)
            nc.vector.tensor_scalar_mul(v_sb[:w, half, DN * n2:DN * (n2 + 1)],
                                        ps[:w, :DN], dsv)


def _attention(c, b, q_fm, k_z, v_sb, o_sb):
    """Transposed-scores attention for one image.

    St[j,i] = sum_d K_z[d,j] Q[d,i] per (jt, head); exp lands straight in
    the aT layout the O matmul wants. Softmax denominators come from a
    ones-column matmul into the same PSUM tile as O (partition-replicated),
    and 1/sum is applied in the fused O copy.
    """
    nc = c.nc
    aT = c.atp.tile([P, 2, H, NTOK], BF, tag="aT")
    for jt in range(2):
        _, jci, jw = _tile_geom(2 * b + jt)
        for hg in range(3):
            sps = c.stp.tile([P, 4, 256], F32, tag="st")
            for hh in range(4):
                h = 4 * hg + hh
                nc.tensor.matmul(
                    sps[:jw, hh, :NTOK],
                    lhsT=k_z[:, h, jci:jci + jw],
                    rhs=q_fm[:, h // 2, NTOK * b:NTOK * (b + 1)],
                    start=True, stop=True)
            nc.scalar.activation(aT[:jw, jt, 4 * hg:4 * hg + 4, :],
                                 sps[:jw, :4, :NTOK], AF.Exp)
    for hp in range(KD):
        pt = c.mmp.tile([P, 2, 256], F32, tag="mm")
        # one accumulation group per 64-partition half: the first (attnV)
        # MM's start marks the half's whole bank zero-region pending, the
        # ones-MM first-writes its own range, then both accumulate.
        for hh in range(2):
            h = 2 * hp + hh
            po = 64 * hh
            for jt in range(2):
                _, _, jw = _tile_geom(2 * b + jt)
                nc.tensor.matmul(
                    pt[po:po + 64, 0, :NTOK],
                    lhsT=v_sb[:jw, jt, DH * h:DH * (h + 1)],
                    rhs=aT[:jw, jt, h, :],
                    start=(jt == 0), stop=(jt == 1), skip_group_check=True)
                nc.tensor.matmul(
                    pt[po:po + 64, 1, :NTOK],
                    lhsT=c.ones64[:jw, :],
                    rhs=aT[:jw, jt, h, :],
                    start=False, stop=(jt == 1), skip_group_check=True)
        rb = c.rbp.tile([P, NTOK], F32, tag="rb")
        nc.vector.reciprocal(rb, pt[:, 1, :NTOK])
        nc.vector.tensor_mul(o_sb[:, hp, :NTOK], pt[:, 0, :NTOK], rb)


def _proj_phase(c, b, wp_sb, o8, dsp):
    nc = c.nc
    for half in range(2):
        t = 2 * b + half
        _, ci, w = _tile_geom(t)
        for n2 in range(2):
            ps = c.mmp.tile([P, 512], F32, tag="mm")
            for kc in range(0, KD, 2):
                nc.tensor.matmul(
                    ps[:w, :DN],
                    lhsT=o8[:, kc:kc + 2, ci:ci + w],
                    rhs=wp_sb[:, kc:kc + 2, DN * n2:DN * (n2 + 1)],
                    start=(kc == 0), stop=(kc == KD - 2), perf_mode=DR)
            tmp = c.htp.tile([P, DN], BF, tag="ptmp")
            if n2 == 0:
                nc.scalar.mul(tmp[:w], ps[:w, :DN], dsp)
            else:
                nc.vector.tensor_scalar_mul(tmp[:w], ps[:w, :DN], dsp)
            dst = c.x_tm[t][:w, DN * n2:DN * (n2 + 1)]
            nc.vector.tensor_add(dst, dst, tmp[:w])


def _mlp(c, wfc_sb, wcp_sb, h_fm):
    nc = c.nc
    for hf in range(2):
        g_sb = c.gsp.tile([P, KM, TN], BF, tag="g")
        for mg in range(KM // 2):
            ps = c.stp.tile([P, 2, 512], F32, tag="st")
            for j in range(2):
                m = 2 * mg + j
                for kc in range(KD):
                    nc.tensor.matmul(
                        ps[:P, j, :TN],
                        lhsT=wfc_sb[:, kc, P * m:P * (m + 1)],
                        rhs=h_fm[:, kc, TN * hf:TN * (hf + 1)],
                        start=(kc == 0), stop=(kc == KD - 1))
            nc.scalar.activation(g_sb[:, 2 * mg:2 * mg + 2, :],
                                 ps[:, :, :TN], AF.Gelu)
        # cproj feature-major: stationary = wcp chunks (full 128 wide, no
        # 69-row token tiles), moving = g (already feature-major). The
        # [768, 394] half-result transposes back token-major for the
        # residual add.
        xcp = c.xcp.tile([P, KD, TN], BF, tag="xcp")
        for dc in range(KD):
            ps = c.mmp.tile([P, 512], F32, tag="mm")
            for kc in range(KM):
                nc.tensor.matmul(
                    ps[:P, :TN],
                    lhsT=wcp_sb[:, kc, P * dc:P * (dc + 1)],
                    rhs=g_sb[:, kc, :],
                    start=(kc == 0), stop=(kc == KM - 1))
            nc.vector.tensor_copy(xcp[:, dc, :], ps[:P, :TN])
        for tl in range(4):
            t = 4 * hf + tl
            b, ci, w = _tile_geom(t)
            ch = NTOK * (b - 2 * hf) + ci
            for g, nk in ((0, 4), (4, 2)):
                tp = c.tpp.tile([P, 4, P], BF, tag="tp")
                for j in range(nk):
                    kc = g + j
                    nc.tensor.transpose(tp[:w, j, :P],
                                        xcp[:, kc, ch:ch + w], c.idb)
                dst = c.x_tm[t][:w, P * g:P * (g + nk)]
                nc.vector.tensor_add(dst, dst, tp[:w, :nk, :])


def _head(c, wproj_d, whead_d, out_d):
    nc = c.nc
    with tc_pool(c, "head", 1) as hd:
        wproj_sb = hd.tile([P, KD, PROJ], BF)
        nc.sync.dma_start(wproj_sb, wproj_d.ap())
        whead_sb = hd.tile([P, PROJ // P, NCLS], BF)
        nc.sync.dma_start(whead_sb, whead_d.ap())
        cls_tm = hd.tile([IMGS, D], BF)
        for b in range(IMGS):
            nc.sync.dma_start(cls_tm[b:b + 1, :], c.x_tm[2 * b][0:1, :])
        clsn = hd.tile([IMGS, D], BF)
        _layernorm_tile(c, cls_tm[:IMGS], clsn[:IMGS], IMGS)
        cls_fm = hd.tile([P, KD, IMGS], BF)
        for kc in range(KD):
            tp = c.tpp.tile([P, 4, P], BF, tag="tp")
            nc.tensor.transpose(tp[:P, 0, :IMGS],
                                clsn[:IMGS, P * kc:P * (kc + 1)],
                                c.idb[:IMGS, :IMGS])
            nc.vector.tensor_copy(cls_fm[:, kc, :], tp[:P, 0, :IMGS])
        z_fm = hd.tile([P, PROJ // P, IMGS], BF)
        for m in range(PROJ // P):
            ps = c.mmp.tile([P, 512], F32, tag="mm")
            for kc in range(KD):
                nc.tensor.matmul(
                    ps[:P, :IMGS],
                    lhsT=wproj_sb[:, kc, P * m:P * (m + 1)],
                    rhs=cls_fm[:, kc, :],
                    start=(kc == 0), stop=(kc == KD - 1))
            nc.vector.tensor_copy(z_fm[:, m, :], ps[:P, :IMGS])
        out_sb = hd.tile([IMGS, NCLS], F32)
        for n2 in range(2):
            ps = c.mmp.tile([P, 512], F32, tag="mm")
            for kc in range(PROJ // P):
                nc.tensor.matmul(
                    ps[:IMGS, :500],
                    lhsT=z_fm[:, kc, :],
                    rhs=whead_sb[:, kc, 500 * n2:500 * (n2 + 1)],
                    start=(kc == 0), stop=(kc == PROJ // P - 1))
            nc.vector.tensor_copy(out_sb[:IMGS, 500 * n2:500 * (n2 + 1)],
                                  ps[:IMGS, :500])
        nc.sync.dma_start(out_d.ap(), out_sb)


def tc_pool(c, name, bufs, space="SBUF"):
    return c.tc.tile_pool(name=name, bufs=bufs, space=space)


def _build(cfg):
    """Build + compile the SPMD Bass program."""
    nc = bacc.Bacc("TRN2", target_bir_lowering=False, debug=False,
                   num_devices=N_CORES)
    depth = cfg["depth"]
    dsq = list(cfg["dsq"])
    dsk = list(cfg["dsk"])
    dsv = list(cfg["dsv"])
    dsp = list(cfg["dsp"])

    patches_d = nc.dram_tensor("patches", [P, KD, IMGS, NTOK], BF,
                               kind="ExternalInput")
    wconv_d = nc.dram_tensor("wconv", [P, KD, D], BF, kind="ExternalInput")
    pos_d = nc.dram_tensor("pos", [P, 2, D], F32, kind="ExternalInput")
    wqk_d, wv_d, wp_d, wfc_d, wcp_d = [], [], [], [], []
    for l in range(depth):
        wqk_d.append(nc.dram_tensor(f"wqk{l}", [P, KD, 2 * D], FP8,
                                    kind="ExternalInput"))
        wv_d.append(nc.dram_tensor(f"wv{l}", [P, KD, D], FP8,
                                   kind="ExternalInput"))
        wp_d.append(nc.dram_tensor(f"wp{l}", [P, KD, D], FP8,
                                   kind="ExternalInput"))
        wfc_d.append(nc.dram_tensor(f"wfc{l}", [P, KD, MLP], BF,
                                    kind="ExternalInput"))
        wcp_d.append(nc.dram_tensor(f"wcp{l}", [P, KM, D], BF,
                                    kind="ExternalInput"))
    wproj_d = nc.dram_tensor("wproj", [P, KD, PROJ], BF, kind="ExternalInput")
    whead_d = nc.dram_tensor("whead", [P, PROJ // P, NCLS], BF,
                             kind="ExternalInput")
    out_d = nc.dram_tensor("out", [IMGS, NCLS], F32, kind="ExternalOutput")

    c = _Ctx()
    c.nc = nc
    c.dbg = bool(cfg.get("dbg"))

    def dump(name, ap):
        if not c.dbg:
            return
        d = nc.dram_tensor(f"dbg_{name}", list(ap.shape), ap.dtype,
                           kind="ExternalOutput")
        nc.sync.dma_start(d.ap(), ap)

    c.dump = dump
    with tile.TileContext(nc) as tc, ExitStack() as st:
        c.tc = tc
        c.const = st.enter_context(tc_pool(c, "const", 1))
        c.xres = st.enter_context(tc_pool(c, "xres", 1))
        c.mmp = st.enter_context(tc_pool(c, "mm", 2, space="PSUM"))
        c.tpp = st.enter_context(tc_pool(c, "tp", 2, space="PSUM"))
        c.stp = st.enter_context(tc_pool(c, "stp", 2, space="PSUM"))
        c.small = st.enter_context(tc_pool(c, "small", 4))
        c.htp = st.enter_context(tc_pool(c, "htile", 2))

        c.idb = c.const.tile([P, P], BF)
        make_identity(nc, c.idb)
        c.epst = c.const.tile([P, 1], F32)
        nc.vector.memset(c.epst, EPS)
        c.ones64 = c.const.tile([P, 64], BF)
        nc.vector.memset(c.ones64, 1.0 / SO)
        c.x_tm = []
        for t in range(NT):
            xt = c.xres.tile([P, D], BF, tag=f"x{t}")
            nc.vector.memset(xt, 0.0)
            c.x_tm.append(xt)
        # persistent zero-padded K buffers (zero halves written once here)
        c.kz_all = c.const.tile([P, 4, H, NTOK], BF)
        nc.vector.memset(c.kz_all, 0.0)

        _embed(c, patches_d, wconv_d, pos_d)
        c.dump("x0", c.x_tm)

        with ExitStack() as ls:
            wqkp = ls.enter_context(tc_pool(c, "wqk", 1))
            wvp = ls.enter_context(tc_pool(c, "wv", 1))
            wpp = ls.enter_context(tc_pool(c, "wp", 1))
            wfcp = ls.enter_context(tc_pool(c, "wfc", 1))
            wcpp = ls.enter_context(tc_pool(c, "wcp", 1))
            c.h8p = ls.enter_context(tc_pool(c, "h8", 1))
            c.hfmp = ls.enter_context(tc_pool(c, "hfm", 1))
            qkp = ls.enter_context(tc_pool(c, "qkfm", 1))
            vip = ls.enter_context(tc_pool(c, "vimg", 4))
            c.atp = ls.enter_context(tc_pool(c, "atT", 2))
            oip = ls.enter_context(tc_pool(c, "oimg", 2))
            c.rbp = ls.enter_context(tc_pool(c, "rb", 2))
            c.gsp = ls.enter_context(tc_pool(c, "gsb", 1))
            c.xcp = ls.enter_context(tc_pool(c, "xcp", 1))

            for l in range(depth):
                wqk_sb = wqkp.tile([P, KD, 2 * D], FP8, tag="wqk")
                nc.sync.dma_start(wqk_sb, wqk_d[l].ap())
                wv_sb = wvp.tile([P, KD, D], FP8, tag="wv")
                nc.sync.dma_start(wv_sb, wv_d[l].ap())
                wp_sb = wpp.tile([P, KD, D], FP8, tag="wp")
                nc.sync.dma_start(wp_sb, wp_d[l].ap())
                wfc_sb = wfcp.tile([P, KD, MLP], BF, tag="wfc")
                nc.sync.dma_start(wfc_sb, wfc_d[l].ap())
                wcp_sb = wcpp.tile([P, KM, D], BF, tag="wcp")
                nc.sync.dma_start(wcp_sb, wcp_d[l].ap())

                h8 = c.h8p.tile([P, KD, TP8], FP8, tag="h8")
                nc.vector.memset(h8[:, :, T:], 0.0)
                _ln_transpose(c, h8, SH)
                if l == 0:
                    c.dump("h8", h8)
                q_fm = qkp.tile([P, KD, T], BF, tag="q")
                _qk_phase(c, wqk_sb, h8, q_fm, dsq[l])
                if l == 0:
                    c.dump("q", q_fm)
                for b in range(IMGS):
                    k_z = c.kz_all[:, b]
                    _k_phase(c, b, wqk_sb, h8, k_z, dsk[l])
                    if l == 0 and b == 0:
                        c.dump("kz", k_z)
                    v_sb = vip.tile([P, 2, D], BF, tag="v")
                    _v_phase(c, b, wv_sb, h8, v_sb, dsv[l])
                    o8 = oip.tile([P, KD, NT8], FP8, tag="o")
                    nc.vector.memset(o8[:, :, NTOK:], 0.0)
                    _attention(c, b, q_fm, k_z, v_sb, o8)
                    if l == 0 and b == 0:
                        c.dump("v0", v_sb)
                        c.dump("o0", o8)
                    _proj_phase(c, b, wp_sb, o8, dsp[l])
                if l == 0:
                    c.dump("xp", c.x_tm)
                h_fm = c.hfmp.tile([P, KD, T], BF, tag="hfm")
                _ln_transpose(c, h_fm, None)
                _mlp(c, wfc_sb, wcp_sb, h_fm)
                if l == 0:
                    c.dump("xl", c.x_tm)

        _head(c, wproj_d, whead_d, out_d)

    nc.compile()
    return nc


def _quant8(a):
    """fp8e4 quantize with a power-of-2 scale; returns (q, scale)."""
    amax = float(np.abs(a).max())
    s = 2.0 ** np.floor(np.log2(240.0 / max(amax, 1e-30)))
    q = np.clip(a * s, -240.0, 240.0).astype(f8e4)
    return q, s


def _prep(inputs, depth=DEPTH):
    """Host-side: fold LN affine into weights, build per-core input maps."""
    f = lambda a: np.asarray(a, np.float32)
    x = f(inputs["x"])
    conv_w = f(inputs["conv_w"])
    cls_token = f(inputs["cls_token"])
    pos_embed = f(inputs["pos_embed"])
    ln_pre_w, ln_pre_b = f(inputs["ln_pre_w"]), f(inputs["ln_pre_b"])
    ln1_w, ln1_b = f(inputs["ln1_w"]), f(inputs["ln1_b"])
    qkv_w, qkv_b = f(inputs["qkv_w"]), f(inputs["qkv_b"])
    pw, pb = f(inputs["attn_proj_w"]), f(inputs["attn_proj_b"])
    ln2_w, ln2_b = f(inputs["ln2_w"]), f(inputs["ln2_b"])
    fc_w, fc_b = f(inputs["fc_w"]), f(inputs["fc_b"])
    cw, cb = f(inputs["cproj_w"]), f(inputs["cproj_b"])
    ln_post_w, ln_post_b = f(inputs["ln_post_w"]), f(inputs["ln_post_b"])
    proj_w = f(inputs["proj_w"])
    head_w, head_b = f(inputs["head_w"]), f(inputs["head_b"])

    B = x.shape[0]
    # im2col: [B, 196, 768] with feature order (c, ph, pw)
    pat = x.reshape(B, 3, 14, 16, 14, 16).transpose(0, 2, 4, 1, 3, 5)
    pat = pat.reshape(B, 196, D)
    # dummy zero patch at token 0 (cls slot), feature-major per image
    pat = np.concatenate([np.zeros((B, 1, D), np.float32), pat], axis=1)
    pat_fm = pat.transpose(0, 2, 1)                       # [B, 768, 197]

    def kxm(a, dt=bf16):  # [K, M] -> [128, K/128, M]
        k, m = a.shape
        return np.ascontiguousarray(
            a.reshape(k // P, P, m).transpose(1, 0, 2)).astype(dt)

    shared = {"wconv": kxm(conv_w.reshape(D, D).T)}
    pos_full = pos_embed[0].copy()                        # [197, 768]
    pos_full[0] += cls_token[0, 0]
    pos_arr = np.zeros((P, 2, D), np.float32)
    pos_arr[:, 0, :] = pos_full[0:128]
    pos_arr[:69, 1, :] = pos_full[128:197]
    shared["pos"] = pos_arr

    dsq, dsk, dsv, dsp = [], [], [], []
    for l in range(depth):
        qw = qkv_w[l] * ln1_w[l][None, :]                 # [2304, 768]
        wq = qw[0:D].T * DH ** -0.5                       # fold attn scale
        wk = qw[D:2 * D].T
        wv = qw[2 * D:3 * D].T
        wq8, sq = _quant8(wq)
        wk8, sk = _quant8(wk)
        wv8, sv = _quant8(wv)
        wp8, sp = _quant8(pw[l].T)
        wqk8 = np.concatenate([wq8.astype(np.float32),
                               wk8.astype(np.float32)], axis=1)
        shared[f"wqk{l}"] = kxm(wqk8, f8e4)
        shared[f"wv{l}"] = kxm(wv8.astype(np.float32), f8e4)
        dsq.append(1.0 / (SH * sq))
        dsk.append(1.0 / (SH * sk))
        dsv.append(1.0 / (SH * sv))
        dsp.append(1.0 / (SO * sp))
        shared[f"wp{l}"] = kxm(wp8.astype(np.float32), f8e4)
        shared[f"wfc{l}"] = kxm((fc_w[l] * ln2_w[l][None, :]).T)
        shared[f"wcp{l}"] = kxm(cw[l].T)
    shared["wproj"] = kxm((proj_w * ln_post_w[None, :]).T)
    shared["whead"] = kxm(head_w.T)

    # effective biases after LN-fold (all zero for the reference inputs)
    qkv_b_eff = qkv_b[:depth] + np.einsum("led,ld->le", qkv_w[:depth],
                                          ln1_b[:depth])
    fc_b_eff = fc_b[:depth] + np.einsum("lmd,ld->lm", fc_w[:depth],
                                        ln2_b[:depth])
    head_b_eff = head_b + head_w @ (proj_w @ ln_post_b)

    cfg = {"depth": depth, "dsq": tuple(dsq), "dsk": tuple(dsk),
           "dsv": tuple(dsv), "dsp": tuple(dsp)}
    nontrivial = [
        not (np.all(ln_pre_w == 1) and np.all(ln_pre_b == 0)),
        bool(np.any(qkv_b_eff != 0)),
        bool(np.any(pb[:depth] != 0)),
        bool(np.any(fc_b_eff != 0)),
        bool(np.any(cb[:depth] != 0)),
        bool(np.any(head_b_eff != 0)),
    ]
    if any(nontrivial):
        raise NotImplementedError(
            "nonzero bias / non-identity LN affine path not built")

    n_per = B // N_CORES
    in_maps = []
    for core in range(N_CORES):
        m = dict(shared)
        pc = pat_fm[core * n_per:(core + 1) * n_per]      # [4, 768, 197]
        m["patches"] = np.ascontiguousarray(
            pc.reshape(n_per, KD, P, NTOK).transpose(2, 1, 0, 3)).astype(bf16)
        in_maps.append(m)
    return in_maps, cfg


_CACHE = {}
LAST_RESULT = None


def kernel(**inputs):
    global LAST_RESULT
    in_maps, cfg = _prep(inputs)
    key = tuple(sorted(cfg.items()))
    if key not in _CACHE:
        _CACHE[key] = _build(cfg)
    nc = _CACHE[key]
    res = run_bass_kernel_spmd(nc, in_maps, core_ids=list(range(N_CORES)))
    LAST_RESULT = res
    return np.concatenate([r["out"] for r in res.results], axis=0)
